# revision 16
# baseline (speedup 1.0000x reference)
"""Trainium2 Bass kernel for nn_CONCATNet_7447473291796 (gnn_message_passing).

Strategy (pure data parallelism, 16 batches per core across 8 cores):
  Only ~66 of the 4096 wafer rows are touched per batch, so the kernel does
  sparse row gathers from HBM-resident bf16 tables via SWDGE dma_gather in
  16-bit transpose mode: each gathered tile lands already transposed
  ([d x rows]) and feeds the PE array directly as the moving operand.

  All matmuls are weight-stationary and compute the output transposed
  ([d_out x entries]) in fp32 PSUM:
    pm.T  = Wcs.T @ stageT + Wcw.T @ waferT + v_dyn ox remain_prs
  streamed as two 512-wide column blocks (one PSUM bank each).  The robot
  arms' loc embeddings are recomputed in-stream as 32 extra columns of the
  same three passes (loc==0 / loc==P+1 specials handled by a mask+const
  fixup), then the arm embedding is three more small weight-stationary
  passes.  No data transposes, no selection matmuls.

  A dummy gather with all-negative (trimmed-to-zero) indices issues right
  after the gpsimd library load to absorb the ucode cold-start before the
  real index tile arrives.  Index padding uses -1, which the ucode strips.

All per-core variation (gather indices, remain_prs, masks) is data staged
through DRAM inputs; the Bass program is identical on every core.
"""

import numpy as np
import ml_dtypes

import concourse.bass as bass
import concourse.bacc as bacc
import concourse.mybir as mybir
import concourse.tile as tile
from concourse import library_config
from concourse.bass_utils import run_bass_kernel_spmd

B, N, S, P, D = 128, 4096, 32, 64, 128
NORM = 300.0
NCORES = 8
BL = B // NCORES          # local batches per core = 16
HALF = BL // 2            # batches per rows table = 8
ZROW = BL * S             # zero-row slot in the col table (=512)
NI = 640                  # num_idxs per gather call (544 used + 96 pad of -1)
NIW = NI // 16            # i16 idx words per partition per call = 40
IDXW = 4 * NIW            # total idx words per partition = 160
NOUT = BL * P + 2 * BL    # 1056 output columns per core

F32 = mybir.dt.float32
BF16 = mybir.dt.bfloat16
U16 = mybir.dt.uint16
U8 = mybir.dt.uint8
I16 = mybir.dt.int16

_prog_cache = None


def _wrap16(idx_flat: np.ndarray) -> np.ndarray:
    """Logical index list -> [128, n//16] int16 SWDGE layout (idx i lives at
    [i % 16, i // 16], replicated into all 8 16-partition groups)."""
    n = idx_flat.shape[0]
    assert n % 16 == 0
    a = idx_flat.astype(np.int16).reshape(n // 16, 16).T
    return np.tile(a, (8, 1))


def _bf16(x: np.ndarray) -> np.ndarray:
    return np.asarray(x, dtype=ml_dtypes.bfloat16).view(np.uint16)


def _build_program():
    nc = bacc.Bacc("TRN2", target_bir_lowering=False, num_swdge_queues=4,
                   debug=False)

    rows0_h = nc.declare_dram_parameter("rows0", [HALF * N, D], U16, isOutput=False)
    rows1_h = nc.declare_dram_parameter("rows1", [HALF * N, D], U16, isOutput=False)
    cols_h = nc.declare_dram_parameter("cols", [ZROW + 1, D], U16, isOutput=False)
    blob_h = nc.declare_dram_parameter("blob", [128, 1856], U8, isOutput=False)
    rv_h = nc.declare_dram_parameter("rv", [1, 2368], U8, isOutput=False)
    out_h = nc.declare_dram_parameter("out", [128, NOUT], F32, isOutput=True)

    with tile.TileContext(nc) as tc:
        with (
            tc.tile_pool(name="consts", bufs=1) as cpool,
            tc.tile_pool(name="gath", bufs=1) as gpool,
            tc.tile_pool(name="outp", bufs=1) as opool,
            tc.tile_pool(name="small", bufs=1) as spool,
            tc.tile_pool(name="psum", bufs=1, space="PSUM") as ppool,
        ):
            nc.gpsimd.load_library(library_config.mlp)

            # ---- constant loads (2 DMAs, parallel queues) ----
            blob = cpool.tile([128, 1856], U8, name="blob")
            nc.sync.dma_start(out=blob[:], in_=blob_h[:])
            rv = cpool.tile([1, 2368], U8, name="rv")
            nc.scalar.dma_start(out=rv[:], in_=rv_h[:])

            idx = blob[:, 0 : 2 * IDXW].bitcast(I16)          # [128, 160]

            # ---- real gathers: cols first (feed pass 1), then rows.
            # 544 valid idxs per call keeps each call within the SWDGE
            # descriptor-ring capacity (74-desc calls hang the rings).
            def gath(tab, word0, q, name):
                g = gpool.tile([128, 1, NI], U16, name=name)
                nc.gpsimd.dma_gather(
                    g[:], tab, idx[:, word0 : word0 + NIW], NI, 544, D,
                    transpose=True, queue_num=q,
                )
                return g
            gc0 = gath(cols_h[:, :], 0, 0, "gc0")
            gc1 = gath(cols_h[:, :], NIW, 1, "gc1")
            g0 = gath(rows0_h[:, :], 2 * NIW, 2, "g0")
            g1 = gath(rows1_h[:, :], 3 * NIW, 3, "g1")

            wst = blob[:, 320:1600].bitcast(BF16)             # [128, 640]
            w_cs = wst[:, 0:128]     # W_concat stage segment   [d_in, d_out]
            w_cw = wst[:, 128:256]   # W_concat wafer segment
            w_rl = wst[:, 256:384]   # W_robot loc segment
            w_rw = wst[:, 384:512]   # W_robot wafer segment
            w_rn = wst[:, 512:640]   # W_robot next-stage segment
            mask = blob[:, 1600:1728].bitcast(F32)            # [128, 32]
            acon = blob[:, 1728:1856].bitcast(F32)            # [128, 32]
            rext = rv[:, 0:2112].bitcast(BF16)                # [1, 1056]
            vdyn = rv[:, 2112:2368].bitcast(BF16)             # [1, 128]

            gc0b = gc0[:, 0, :].bitcast(BF16)                 # [128, 640]
            gc1b = gc1[:, 0, :].bitcast(BF16)
            g0b = g0[:, 0, :].bitcast(BF16)
            g1b = g1[:, 0, :].bitcast(BF16)

            p0 = ppool.tile([128, 512], F32, name="p0", tag="p0")
            p1 = ppool.tile([128, 512], F32, name="p1", tag="p1")
            pl0 = ppool.tile([128, 16], F32, name="pl0", tag="pl0")
            pl1 = ppool.tile([128, 16], F32, name="pl1", tag="pl1")
            pa0 = ppool.tile([128, 16], F32, name="pa0", tag="pa0")
            pa1 = ppool.tile([128, 16], F32, name="pa1", tag="pa1")
            out_sb = opool.tile([128, NOUT], F32, name="out_sb")

            mm = nc.tensor.matmul

            # ---- keep the PE busy while gathers fly so it ramps to full
            # p-state before the real matmuls (idle PE runs at half clock)
            warmp = ppool.tile([128, 512], F32, name="warmp", tag="warmp")
            for _ in range(24):
                mm(warmp[:], lhsT=w_cs, rhs=wst[:, 0:512], start=True,
                   stop=True)

            # ---- pm.T half 0 (finishes as soon as cols+rows0 land) ----
            mm(p0[:], lhsT=w_cs, rhs=gc0b[:, 0:512], start=True, stop=False)
            mm(p0[:], lhsT=w_cw, rhs=g0b[:, 0:512], start=False, stop=False)
            mm(p0[:], lhsT=vdyn, rhs=rext[:, 0:512], start=False, stop=True)
            nc.vector.tensor_copy(out=out_sb[:, 0:512], in_=p0[:])
            nc.sync.dma_start(out=out_h[:, 0:512], in_=out_sb[:, 0:512])

            # ---- pm.T half 1 ----
            mm(p1[:], lhsT=w_cs, rhs=gc1b[:, 0:512], start=True, stop=False)
            mm(p1[:], lhsT=w_cw, rhs=g1b[:, 0:512], start=False, stop=False)
            mm(p1[:], lhsT=vdyn, rhs=rext[:, 512:1024], start=False, stop=True)
            nc.vector.tensor_copy(out=out_sb[:, 512:1024], in_=p1[:])
            nc.sync.dma_start(out=out_h[:, 512:1024], in_=out_sb[:, 512:1024])

            # ---- arm a_loc columns (recomputed pm entries), one psum/half ----
            mm(pl0[:], lhsT=w_cs, rhs=gc0b[:, 512:528], start=True, stop=False)
            mm(pl0[:], lhsT=w_cw, rhs=g0b[:, 528:544], start=False, stop=False)
            mm(pl0[:], lhsT=vdyn, rhs=rext[:, 1024:1040], start=False, stop=True)
            mm(pl1[:], lhsT=w_cs, rhs=gc1b[:, 512:528], start=True, stop=False)
            mm(pl1[:], lhsT=w_cw, rhs=g1b[:, 528:544], start=False, stop=False)
            mm(pl1[:], lhsT=vdyn, rhs=rext[:, 1040:1056], start=False, stop=True)
            # special locs: col *= mask (0/1), += const (ones row for loc==P+1)
            alo = spool.tile([128, 32], BF16, name="alo")
            alof = spool.tile([128, 32], F32, name="alof")
            nc.vector.tensor_tensor(out=alof[:, 0:16], in0=pl0[:],
                                    in1=mask[:, 0:16], op=mybir.AluOpType.mult)
            nc.vector.tensor_tensor(out=alof[:, 16:32], in0=pl1[:],
                                    in1=mask[:, 16:32], op=mybir.AluOpType.mult)
            nc.vector.tensor_tensor(out=alo[:], in0=alof[:], in1=acon,
                                    op=mybir.AluOpType.add)

            # ---- arm embedding, one psum/half ----
            mm(pa0[:], lhsT=w_rl, rhs=alo[:, 0:16], start=True, stop=False)
            mm(pa0[:], lhsT=w_rw, rhs=g0b[:, 512:528], start=False, stop=False)
            mm(pa0[:], lhsT=w_rn, rhs=gc0b[:, 528:544], start=False, stop=True)
            mm(pa1[:], lhsT=w_rl, rhs=alo[:, 16:32], start=True, stop=False)
            mm(pa1[:], lhsT=w_rw, rhs=g1b[:, 512:528], start=False, stop=False)
            mm(pa1[:], lhsT=w_rn, rhs=gc1b[:, 528:544], start=False, stop=True)
            nc.vector.tensor_copy(out=out_sb[:, 1024:1040], in_=pa0[:])
            nc.vector.tensor_copy(out=out_sb[:, 1040:1056], in_=pa1[:])
            nc.sync.dma_start(out=out_h[:, 1024:1056], in_=out_sb[:, 1024:1056])

    nc.compile()
    return nc


def _get_program():
    global _prog_cache
    if _prog_cache is None:
        _prog_cache = _build_program()
    return _prog_cache


def _prep_core(c, encoded_row, encoded_col, clock, loc_process_end_time,
               W_dyn, W_concat, W_robot, loc_hold_wafer, loc_stage,
               robot_arm1_loc, robot_arm2_loc, arm1_recipe, arm2_recipe,
               arm1_next_stage, arm2_next_stage, wblob_part, v_dyn_bf):
    b0 = c * BL
    bs = slice(b0, b0 + BL)

    rows = _bf16(encoded_row[bs].reshape(BL * N, D))
    rows0 = np.ascontiguousarray(rows[0 : HALF * N])
    rows1 = np.ascontiguousarray(rows[HALF * N :])
    cols = np.concatenate(
        [_bf16(encoded_col[bs].reshape(BL * S, D)),
         np.zeros((1, D), np.uint16)], axis=0)

    lhw = np.maximum(loc_hold_wafer[bs].astype(np.int64), 0)      # [16, 64]
    lst = loc_stage[bs].astype(np.int64)                          # [16, 64]
    rec = np.maximum(np.stack([arm1_recipe[bs, 0], arm2_recipe[bs, 0]],
                              axis=1).astype(np.int64), 0)        # [16, 2]
    nst = np.stack([arm1_next_stage[bs, 0], arm2_next_stage[bs, 0]],
                   axis=1).astype(np.int64)
    loc = np.stack([robot_arm1_loc[bs, 0], robot_arm2_loc[bs, 0]],
                   axis=1).astype(np.int64)
    remain = np.maximum(
        loc_process_end_time[bs] - clock[bs], 0.0).astype(np.float32) / NORM

    lb16 = np.arange(BL)
    lb8 = np.arange(HALF)
    normal = (loc >= 1) & (loc <= P)                              # [16, 2]
    locc = np.where(normal, loc, 1)                               # safe loc-1

    # cols call: [stage_h0 512 | stage_h1 512 | locstage 32 | ns 32 | pad 64]
    stage_all = (lb16[:, None] * S + (lst - 1))                   # [16, 64]
    locstage = np.where(
        normal, lb16[:, None] * S
        + np.take_along_axis(lst, locc - 1, axis=1) - 1, ZROW)    # [16, 2]
    ns_all = np.where((nst >= 1) & (nst <= S),
                      lb16[:, None] * S + nst - 1, ZROW)          # [16, 2]
    # cols call h: [stage 512 | locstage 16 | ns 16 | pad 96]
    locwafer = np.where(normal, np.take_along_axis(lhw, locc - 1, axis=1), 0)
    pad = np.full(NI - HALF * P - 4 * HALF, -1, np.int64)
    calls = []
    for h in range(2):
        hb = slice(h * HALF, (h + 1) * HALF)
        calls.append(np.concatenate([
            stage_all[hb].reshape(-1), locstage[hb].reshape(-1),
            ns_all[hb].reshape(-1), pad]))
    # rows call h: [wafer 512 | recipe 16 | locwafer 16 | pad 96]
    for h in range(2):
        hb = slice(h * HALF, (h + 1) * HALF)
        calls.append(np.concatenate([
            (lb8[:, None] * N + lhw[hb]).reshape(-1),
            (lb8[:, None] * N + rec[hb]).reshape(-1),
            (lb8[:, None] * N + locwafer[hb]).reshape(-1), pad]))
    idx = np.concatenate([_wrap16(c) for c in calls], axis=1)     # [128, 160]

    maskv = normal.reshape(-1).astype(np.float32)                 # [32]
    aconv = (loc == P + 1).reshape(-1).astype(np.float32)
    blob = np.empty((128, 1856), np.uint8)
    blob[:, 0:320] = idx.view(np.uint8).reshape(128, 320)
    blob[:, 320:1600] = wblob_part
    blob[:, 1600:1728] = np.broadcast_to(
        maskv.view(np.uint8).reshape(1, 128), (128, 128))
    blob[:, 1728:1856] = np.broadcast_to(
        aconv.view(np.uint8).reshape(1, 128), (128, 128))

    r_arm = np.where(normal,
                     np.take_along_axis(remain, locc - 1, axis=1), 0.0)
    rext = np.concatenate([remain.reshape(-1), r_arm.reshape(-1)])  # [1056]
    rv = np.empty((1, 2368), np.uint8)
    rv[0, 0:2112] = _bf16(rext).view(np.uint8)
    rv[0, 2112:2368] = v_dyn_bf.view(np.uint8)

    return {"rows0": rows0, "rows1": rows1, "cols": cols,
            "blob": blob, "rv": rv}


def make_in_maps(inputs):
    inputs = {k: np.asarray(v) for k, v in inputs.items()}
    W_concat = inputs["W_concat"].astype(np.float32)
    W_robot = inputs["W_robot"].astype(np.float32)
    W_dyn = inputs["W_dyn"].astype(np.float32)

    # [d_in, 5*d_out] bf16: Wcs | Wcw | Wrl | Wrw | Wrn
    wstack = np.concatenate(
        [W_concat[0:D], W_concat[D : 2 * D],
         W_robot[0:D], W_robot[D : 2 * D], W_robot[2 * D : 3 * D]], axis=1)
    wblob_part = _bf16(wstack).view(np.uint8).reshape(128, 1280)
    v_dyn_bf = _bf16((W_dyn[0:1] @ W_concat[2 * D : 3 * D]).reshape(D))

    return [
        _prep_core(c, wblob_part=wblob_part, v_dyn_bf=v_dyn_bf, **inputs)
        for c in range(NCORES)
    ]


def assemble_output(res):
    out = np.empty((B, P + 2, D), np.float32)
    for c in range(NCORES):
        r = res[c]["out"]                                 # [128, 1056]
        pm = r[:, 0:1024].reshape(D, BL, P).transpose(1, 2, 0)
        arm = r[:, 1024:1056].reshape(D, BL, 2).transpose(1, 2, 0)
        out[c * BL : (c + 1) * BL, 0:P, :] = pm
        out[c * BL : (c + 1) * BL, P:, :] = arm
    return out


def kernel(**inputs):
    in_maps = make_in_maps(inputs)
    nc = _get_program()
    res = run_bass_kernel_spmd(nc, in_maps, list(range(NCORES))).results
    return assemble_output(res)
